# revision 1
# baseline (speedup 1.0000x reference)
"""BiTreeLSTM forward over a complete binary tree (N=8192, F=2048, H=1024),
distributed over 8 trn2 NeuronCores.

Strategy: the tree with parent(t) = (t-1)//2 has contiguous levels; level d
(nodes [2^d-1, 2^{d+1}-2]) splits into 8 contiguous chunks whose parents are
exactly the previous level's chunks of the same rank — i.e. 8 independent
subtrees rooted at the level-3 nodes 7..14.  The host computes the 15-node
prefix (levels 0..3, 0.2% of the work), then each core runs one subtree:

  phase A: xproj = feats @ Wx.T + (bx+bh) for its 1023 nodes (f32r matmuls)
  phase B: 10 levelized LSTM steps (levels 4..12 chunk + node 8191):
           gates = xproj + Hpar @ Wh.T (bf16 matmuls, stationary = HparT,
           moving = WhT, 8x stationary reuse), fp32 elementwise, parent
           gather by strided DMA, HparT built via PE transpose + DVE
           repeated-column copy.

No cross-core communication.  Host scatters per-core inputs / gathers rows.
"""
import os
import sys
import math

import numpy as np

for _p in ("/opt/trn_rl_repo", "/root/.axon_site/_ro/trn_rl_repo"):
    if os.path.isdir(_p) and _p not in sys.path:
        sys.path.insert(0, _p)

N = 8192
F = 2048
H = 1024
G = 4096
NCORES = 8
# levels handled on-device per core: (level, size, local_offset)
LEVELS = [(d, 1 << (d - 3), (1 << (d - 3)) - 2) for d in range(4, 13)] + [(13, 1, 1022)]
LOCAL_N = 1024  # 1022 level nodes + node 8191 + 1 pad

_prog_cache = {}


# ----------------------------------------------------------------- reference
def _sigmoid(x):
    out = np.empty_like(x)
    np.negative(x, out=out)
    np.exp(out, out=out)
    out += 1.0
    np.reciprocal(out, out=out)
    return out


def _lstm_batch(xp, hp, cp, WhT):
    """One batched node update: returns (h, c). All fp32."""
    iofu = xp + hp @ WhT
    i, o, f, u = np.split(iofu, 4, axis=1)
    i = _sigmoid(i)
    o = _sigmoid(o)
    f = _sigmoid(f)
    u = np.tanh(u)
    c = i * u + f * cp
    h = o * np.tanh(c)
    return h, c


def _numpy_fallback(features, Wx, bx, Wh, bh, parent_idx, root_c, root_h):
    """Exact levelized numpy evaluation for arbitrary parent_idx.

    Matches reference semantics: at step t the buffer slot p+1 holds the
    root state (p==-1), node p's state (p < t) or zeros (p >= t).
    """
    n = features.shape[0]
    xproj = features @ Wx.T + (bx + bh)
    WhT = Wh.T.copy()
    lvl = np.zeros(n, np.int64)
    p = parent_idx
    for t in range(n):
        pt = p[t]
        lvl[t] = 0 if (pt < 0 or pt >= t) else lvl[pt] + 1
    hidden = np.zeros((n, H), np.float32)
    c_all = np.zeros((n, H), np.float32)
    for lv in range(int(lvl.max()) + 1):
        nodes = np.where(lvl == lv)[0]
        pn = p[nodes]
        hp = np.where((pn >= 0)[:, None],
                      np.where((pn < nodes)[:, None], hidden[pn], 0.0),
                      root_h)
        cp = np.where((pn >= 0)[:, None],
                      np.where((pn < nodes)[:, None], c_all[pn], 0.0),
                      root_c)
        h, c = _lstm_batch(xproj[nodes], hp.astype(np.float32),
                           cp.astype(np.float32), WhT)
        hidden[nodes] = h
        c_all[nodes] = c
    return hidden


# ------------------------------------------------------------ device program
def _build_program(include_a=True, include_b=True, loop_n=None,
                   xproj_external=False):
    import concourse.bass as bass
    import concourse.mybir as mybir
    import concourse.tile as tile
    from concourse.masks import make_identity
    from contextlib import ExitStack

    F32 = mybir.dt.float32
    F32R = mybir.dt.float32r
    BF16 = mybir.dt.bfloat16
    AF = mybir.ActivationFunctionType

    nc = bass.Bass(target_bir_lowering=False)

    if include_a:
        feats_t = nc.declare_dram_parameter("feats_t", [F, LOCAL_N], F32R, isOutput=False)
        wxt = nc.declare_dram_parameter("wxt", [4, 16, 128, 1024], F32R, isOutput=False)
        biasv = nc.declare_dram_parameter("biasv", [G], F32, isOutput=False)
    if include_b:
        wht_bf = nc.declare_dram_parameter("wht_bf", [H, G], BF16, isOutput=False)
        hparT4 = nc.declare_dram_parameter("hparT4", [128, 8, 2], BF16, isOutput=False)
        cp4 = nc.declare_dram_parameter("cp4", [2, H], F32, isOutput=False)
        hid = nc.declare_dram_parameter("hid", [LOCAL_N, H], F32, isOutput=True)
    if xproj_external:
        if include_a and not include_b:
            xproj_d = nc.declare_dram_parameter("xproj", [LOCAL_N, G], F32, isOutput=True)
        else:
            xproj_d = nc.declare_dram_parameter("xproj", [LOCAL_N, G], F32, isOutput=False)
    else:
        xproj_d = None  # allocated as a DRAM pool tile inside the TileContext

    if loop_n is not None and loop_n > 1:
        assert not (include_a and include_b), "loop mode is single-phase only"

    with tile.TileContext(nc) as tc:
        with ExitStack() as stack:
            persist = stack.enter_context(tc.tile_pool(name="persist", bufs=1))
            psp = stack.enter_context(tc.tile_pool(name="psp", bufs=8, space="PSUM"))
            if xproj_d is None:
                pdram = stack.enter_context(tc.tile_pool(name="pdram", bufs=1, space="DRAM"))
                xproj_d = pdram.tile([LOCAL_N, G], F32, name="xproj_t")

            ident = persist.tile([128, 128], F32, name="ident")
            make_identity(nc, ident[:])

            def emit_phase_a(loop_iv=None):
                ft = pa_feat.tile([128, 16, LOCAL_N], F32R, name="ft")
                for k in range(16):
                    eng = nc.sync if k % 2 == 0 else nc.scalar
                    eng.dma_start(ft[:, k, :], feats_t[k * 128:(k + 1) * 128, :])
                bfull = pa_feat.tile([128, G], F32, name="bfull")
                bsrc = biasv[:]
                nc.sync.dma_start(bfull[:], bass.AP(tensor=bsrc.tensor, offset=bsrc.offset,
                                                    ap=[[0, 128], bsrc.ap[0]]))
                for mb in range(2):          # node blocks of 512
                    stm = [pa_st.tile([128, G], F32, name=f"st{mb}{m}", tag="st")
                           for m in range(4)]
                    for nb in range(4):      # gate-column blocks of 1024
                        ps = [psp.tile([128, 512], F32, name=f"pa{nb}{mb}{j}", tag="ps")
                              for j in range(8)]
                        for k in range(16):
                            wxk = pa_wx.tile([128, 1024], F32R, name=f"wx{nb}{mb}{k}", tag="wx")
                            eng = nc.sync if k % 2 == 0 else nc.scalar
                            eng.dma_start(wxk[:], wxt[nb, k])
                            for m in range(4):
                                for n in range(2):
                                    nc.tensor.matmul(
                                        ps[m * 2 + n][:],
                                        ft[:, k, (mb * 4 + m) * 128:(mb * 4 + m + 1) * 128],
                                        wxk[:, n * 512:(n + 1) * 512],
                                        start=(k == 0), stop=(k == 15))
                        for m in range(4):
                            for n in range(2):
                                col0 = nb * 1024 + n * 512
                                nc.vector.tensor_add(stm[m][:, col0:col0 + 512],
                                                     ps[m * 2 + n][:],
                                                     bfull[:, col0:col0 + 512])
                    for m in range(4):
                        row0 = (mb * 4 + m) * 128
                        if loop_n is not None and loop_n > 1:
                            eng = nc.sync if m % 2 == 0 else nc.scalar
                        else:
                            eng = nc.gpsimd
                        eng.dma_start(xproj_d[row0:row0 + 128, :], stm[m][:])

            def emit_phase_b(loop_iv=None):
                whtb = pb_w.tile([128, 8, G], BF16, name="whtb")
                for kc in range(8):
                    eng = nc.sync if kc % 2 == 0 else nc.scalar
                    eng.dma_start(whtb[:, kc, :], wht_bf[kc * 128:(kc + 1) * 128, :])
                hpar_cur = pb_hpar.tile([128, 8, 2], BF16, name="hpar4", tag="hpar")
                nc.sync.dma_start(hpar_cur[:], hparT4[:])
                c_prev = None
                for (lvl, nd, off) in LEVELS:
                    nch = max(1, (nd + 127) // 128)
                    c_new = pb_c.tile([128, max(1, nch), H], F32, name=f"c{lvl}", tag="c")
                    nxt = 2 * nd if lvl < 12 else (1 if lvl == 12 else 0)
                    hpar_next = None
                    if nxt:
                        hpar_next = pb_hpar.tile([128, 8, nxt], BF16,
                                                 name=f"hpar{lvl + 1}", tag="hpar")
                    for mc in range(nch):
                        mm = min(128, nd - mc * 128)
                        c0 = mc * 128
                        ps = [psp.tile([128, 512], F32, name=f"g{lvl}_{mc}_{n}", tag="ps")
                              for n in range(8)]
                        for k in range(8):
                            for n in range(8):
                                nc.tensor.matmul(
                                    ps[n][:mm, :],
                                    hpar_cur[:, k, c0:c0 + mm],
                                    whtb[:, k, n * 512:(n + 1) * 512],
                                    start=(k == 0), stop=(k == 7))
                        xp = pb_xp.tile([128, G], F32, name=f"xp{lvl}_{mc}", tag="xp")
                        xp_eng = nc.sync if mc % 2 == 0 else nc.scalar
                        xp_eng.dma_start(xp[:mm, :], xproj_d[off + c0:off + c0 + mm, :])
                        if lvl == 4:
                            cp = pb_cp.tile([128, H], F32, name="cp4t", tag="cp")
                            nc.sync.dma_start(cp[:2, :], cp4[:])
                        elif lvl == 13:
                            cp = None
                        else:
                            cp = pb_cp.tile([128, H], F32, name=f"cp{lvl}_{mc}", tag="cp")
                            pc, r0, cnt = mc // 2, (mc * 64) % 128, mm // 2
                            nc.sync.dma_start(cp[0:mm:2, :], c_prev[r0:r0 + cnt, pc, :])
                            nc.sync.dma_start(cp[1:mm:2, :], c_prev[r0:r0 + cnt, pc, :])
                        hch = pb_h.tile([128, H], F32, name=f"h{lvl}_{mc}", tag="h")
                        for h2 in range(2):
                            cs = h2 * 512
                            gs = {}
                            for gx, (gi_, gate) in enumerate(zip((0, 2, 4, 6), "iofu")):
                                g = pb_g.tile([128, 512], F32,
                                              name=f"g{lvl}_{mc}_{gate}{h2}", tag="g")
                                xcol = gx * 1024 + cs
                                nc.vector.tensor_add(g[:mm, :], ps[gi_ + h2][:mm, :],
                                                     xp[:mm, xcol:xcol + 512])
                                nc.scalar.activation(
                                    out=g[:mm, :], in_=g[:mm, :],
                                    func=AF.Tanh if gate == "u" else AF.Sigmoid)
                                gs[gate] = g
                            t1 = pb_g.tile([128, 512], F32, name=f"t1{lvl}_{mc}_{h2}", tag="g")
                            nc.vector.tensor_mul(t1[:mm, :], gs["i"][:mm, :], gs["u"][:mm, :])
                            if lvl == 13:
                                cpin = c_prev[0:1, 0, cs:cs + 512]
                            else:
                                cpin = cp[:mm, cs:cs + 512]
                            t2 = pb_g.tile([128, 512], F32, name=f"t2{lvl}_{mc}_{h2}", tag="g")
                            nc.vector.tensor_mul(t2[:mm, :], gs["f"][:mm, :], cpin)
                            nc.vector.tensor_add(c_new[:mm, mc, cs:cs + 512],
                                                 t1[:mm, :], t2[:mm, :])
                            tct = pb_g.tile([128, 512], F32, name=f"tc{lvl}_{mc}_{h2}", tag="g")
                            nc.scalar.activation(out=tct[:mm, :],
                                                 in_=c_new[:mm, mc, cs:cs + 512],
                                                 func=AF.Tanh)
                            nc.vector.tensor_mul(hch[:mm, cs:cs + 512],
                                                 gs["o"][:mm, :], tct[:mm, :])
                        nc.sync.dma_start(hid[off + c0:off + c0 + mm, :], hch[:mm, :])
                        # build HparT for the next level from this chunk's h
                        if hpar_next is not None and (lvl < 12 or mc == 0):
                            rep = 2 if lvl < 12 else 1
                            w = mm if lvl < 12 else 1
                            for k in range(8):
                                pT = psp.tile([128, 512], F32,
                                              name=f"pT{lvl}_{mc}_{k}", tag="ps")
                                nc.tensor.transpose(pT[:, :w], hch[:w, k * 128:(k + 1) * 128],
                                                    ident[:w, :w])
                                src = pT[:, :w]
                                if rep == 2:
                                    rap = bass.AP(tensor=src.tensor, offset=src.offset,
                                                  ap=[src.ap[0], src.ap[1], [0, 2]])
                                    dst = hpar_next[:, k, 2 * c0:2 * (c0 + mm)]
                                    nc.vector.tensor_copy(
                                        dst.rearrange("p (n two) -> p n two", two=2), rap)
                                else:
                                    nc.vector.tensor_copy(hpar_next[:, k, 0:1], src)
                    c_prev = c_new
                    hpar_cur = hpar_next

            if include_a:
                with tc.tile_pool(name="pa_feat", bufs=1) as pa_feat, \
                     tc.tile_pool(name="pa_wx", bufs=5) as pa_wx, \
                     tc.tile_pool(name="pa_st", bufs=5) as pa_st:
                    if loop_n is not None and loop_n > 1:
                        with tc.For_i(0, loop_n, 1) as iv:
                            emit_phase_a(iv)
                    else:
                        emit_phase_a()
            if include_b:
                with tc.tile_pool(name="pb_w", bufs=1) as pb_w, \
                     tc.tile_pool(name="pb_hpar", bufs=2) as pb_hpar, \
                     tc.tile_pool(name="pb_c", bufs=2) as pb_c, \
                     tc.tile_pool(name="pb_xp", bufs=2) as pb_xp, \
                     tc.tile_pool(name="pb_cp", bufs=2) as pb_cp, \
                     tc.tile_pool(name="pb_g", bufs=10) as pb_g, \
                     tc.tile_pool(name="pb_h", bufs=3) as pb_h:
                    if loop_n is not None and loop_n > 1:
                        with tc.For_i(0, loop_n, 1) as iv:
                            emit_phase_b(iv)
                    else:
                        emit_phase_b()

    _split_excess_waits(nc)
    return nc


def _split_excess_waits(nc, max_waits=1):
    """This walrus build rejects >1 sem wait per hardware instruction; spill
    the excess onto same-engine NoOps placed immediately before."""
    import concourse.mybir as mybir
    ctr = 0
    for fn in nc.m.functions:
        for bb in fn.blocks:
            il = bb.instructions
            if not any(i.sync_info is not None and i.sync_info.on_wait
                       and len(i.sync_info.on_wait) > max_waits for i in il):
                continue
            new_list = []
            for inst in il:
                si = inst.sync_info
                if si is not None and si.on_wait and len(si.on_wait) > max_waits:
                    waits = list(si.on_wait)
                    for w in waits[:-max_waits]:
                        ctr += 1
                        nop = mybir.InstNoOp(name=f"waitspill_{ctr}", ins=[], outs=[])
                        nop.engine = inst.engine
                        nop.sync_info = mybir.SyncInfo(on_wait=[w], on_update=[])
                        try:
                            nc.register_instruction(nop, overwrite=True)
                        except Exception:
                            pass
                        new_list.append(nop)
                    si.on_wait = waits[-max_waits:]
                new_list.append(inst)
            bb.instructions[:] = new_list
    return ctr


# ------------------------------------------------------------------ host side
def _block_wxt(Wx):
    """WxT [F, G] -> contiguous DMA slabs [nb, k, 128, 1024]."""
    wxt = Wx.T.reshape(16, 128, 4, 1024)
    return np.ascontiguousarray(wxt.transpose(2, 0, 1, 3))


def _host_prefix(features, Wx, bx, Wh, bh, root_c, root_h):
    """Compute nodes 0..14 (levels 0..3) on the host. Returns (h15, c15)."""
    xp = features[0:15] @ Wx.T + (bx + bh)
    WhT = np.ascontiguousarray(Wh.T)
    h15 = np.zeros((15, H), np.float32)
    c15 = np.zeros((15, H), np.float32)
    groups = [([0], None), ([1, 2], [0, 0]), ([3, 4, 5, 6], [1, 1, 2, 2]),
              ([7, 8, 9, 10, 11, 12, 13, 14], [3, 3, 4, 4, 5, 5, 6, 6])]
    for nodes, pars in groups:
        if pars is None:
            hp = np.repeat(root_h.reshape(1, H), len(nodes), 0)
            cp = np.repeat(root_c.reshape(1, H), len(nodes), 0)
        else:
            hp, cp = h15[pars], c15[pars]
        h, c = _lstm_batch(xp[nodes], hp, cp, WhT)
        h15[nodes] = h
        c15[nodes] = c
    return h15, c15


def _local_rows(core):
    rows = []
    for (d, s, off) in LEVELS[:-1]:
        g0 = (1 << d) - 1 + core * s
        rows.append(np.arange(g0, g0 + s))
    rows.append(np.array([8191]))
    rows.append(np.array([0]))  # pad
    return np.concatenate(rows)


def _make_core_inputs(features_t_full, h15, c15, shared):
    """Build the 8 per-core in_maps. features_t_full is [F, N] fp32."""
    import ml_dtypes
    maps = []
    for k in range(NCORES):
        rows = _local_rows(k)
        ft = np.ascontiguousarray(features_t_full[:, rows])
        hr = h15[7 + k]
        cr = c15[7 + k]
        hparT4 = np.ascontiguousarray(
            np.repeat(hr.reshape(8, 128).T[:, :, None], 2, axis=2)
        ).astype(ml_dtypes.bfloat16)
        cp4 = np.ascontiguousarray(np.repeat(cr.reshape(1, H), 2, axis=0))
        m = dict(shared)
        m["feats_t"] = ft
        m["hparT4"] = hparT4
        m["cp4"] = cp4
        maps.append(m)
    return maps


def _assemble(h15, results):
    hidden = np.empty((N, H), np.float32)
    hidden[0:15] = h15
    for k in range(NCORES):
        hk = results[k]["hid"]
        for (d, s, off) in LEVELS[:-1]:
            g0 = (1 << d) - 1 + k * s
            hidden[g0:g0 + s] = hk[off:off + s]
    hidden[8191] = results[0]["hid"][1022]
    return hidden


def _expected_parent_idx():
    t = np.arange(N)
    p = (t - 1) // 2
    p[0] = -1
    return p.astype(np.int64)


def kernel(features, Wx, bx, Wh, bh, parent_idx, root_c, root_h):
    features = np.ascontiguousarray(np.asarray(features, dtype=np.float32))
    Wx = np.ascontiguousarray(np.asarray(Wx, dtype=np.float32))
    bx = np.asarray(bx, dtype=np.float32)
    Wh = np.ascontiguousarray(np.asarray(Wh, dtype=np.float32))
    bh = np.asarray(bh, dtype=np.float32)
    parent_idx = np.asarray(parent_idx)
    root_c = np.asarray(root_c, dtype=np.float32)
    root_h = np.asarray(root_h, dtype=np.float32)

    if (features.shape != (N, F) or Wx.shape != (G, F) or Wh.shape != (G, H)
            or not np.array_equal(parent_idx.astype(np.int64).ravel(),
                                  _expected_parent_idx())):
        return _numpy_fallback(features, Wx, bx, Wh, bh,
                               parent_idx.astype(np.int64).ravel(),
                               root_c.reshape(1, -1), root_h.reshape(1, -1))

    try:
        return _device_kernel(features, Wx, bx, Wh, bh, root_c, root_h)
    except Exception as e:  # device flake → retry once, then fall back
        sys.stderr.write(f"[kernel] device path failed ({type(e).__name__}: {e}); retrying\n")
        try:
            return _device_kernel(features, Wx, bx, Wh, bh, root_c, root_h)
        except Exception as e2:
            sys.stderr.write(f"[kernel] device retry failed ({type(e2).__name__}: {e2}); "
                             "using numpy fallback\n")
            return _numpy_fallback(features, Wx, bx, Wh, bh,
                                   parent_idx.astype(np.int64).ravel(),
                                   root_c.reshape(1, -1), root_h.reshape(1, -1))


def _device_kernel(features, Wx, bx, Wh, bh, root_c, root_h):
    import ml_dtypes
    from concourse.bass_utils import run_bass_kernel_spmd

    h15, c15 = _host_prefix(features, Wx, bx, Wh, bh, root_c, root_h)

    if "main" not in _prog_cache:
        _prog_cache["main"] = _build_program()
    nc = _prog_cache["main"]

    features_t = np.ascontiguousarray(features.T)
    shared = {
        "wxt": _block_wxt(Wx),
        "biasv": np.ascontiguousarray(bx + bh),
        "wht_bf": np.ascontiguousarray(Wh.T).astype(ml_dtypes.bfloat16),
    }
    in_maps = _make_core_inputs(features_t, h15, c15, shared)
    results = run_bass_kernel_spmd(nc, in_maps, list(range(NCORES))).results
    return _assemble(h15, results)



# revision 3
# speedup vs baseline: 1.0579x; 1.0579x over previous
"""BiTreeLSTM forward, v2: fused single program per core, bf16 datapath,
phase-A GEMM interleaved into phase-B level-recurrence stalls.

Tree split: host computes levels 0-6 (127 nodes, 1.5% of FLOPs — the
latency-dominated treetop) and node 8191 (post-pass); each of 8 cores runs
one depth-6 subtree (levels 7-12, 1008 nodes).

Device program (per core), storage row = local + 16 (levels 10+ align to
128-row chunks; chunk 0 = 16 pad rows + levels 7-9):
  A-head: xproj for storage chunk 0 -> DRAM xpc[0], top priority.
  A-main: xproj chunks 1-7, LOW priority -> PE filler during B stalls.
  B: 9 level-chunks of gates = xp + hpar @ WhT (bf16 matmuls, stationary =
     parent-h transposed replicated x2), bf16 eltwise, bf16 c-state,
     bf16 PE transpose to build the next level's stationary.
"""
import os
import sys
import math

import numpy as np

for _p in ("/opt/trn_rl_repo", "/root/.axon_site/_ro/trn_rl_repo"):
    if os.path.isdir(_p) and _p not in sys.path:
        sys.path.insert(0, _p)

N = 8192
F = 2048
H = 1024
G = 4096
NCORES = 8
# device levels: (level, per-core size, local offset); locals 0..1007
LEVELS = [(d, 1 << (d - 3), (1 << (d - 3)) - 16) for d in range(7, 13)]
NLOC = 1008
SHIFT = 16  # storage row = local + SHIFT

_prog_cache = {}


# ----------------------------------------------------------------- reference
def _sigmoid(x):
    out = np.empty_like(x)
    np.negative(x, out=out)
    np.exp(out, out=out)
    out += 1.0
    np.reciprocal(out, out=out)
    return out


def _lstm_batch(xp, hp, cp, WhT):
    iofu = xp + hp @ WhT
    i, o, f, u = np.split(iofu, 4, axis=1)
    i = _sigmoid(i)
    o = _sigmoid(o)
    f = _sigmoid(f)
    u = np.tanh(u)
    c = i * u + f * cp
    h = o * np.tanh(c)
    return h, c


def _numpy_fallback(features, Wx, bx, Wh, bh, parent_idx, root_c, root_h):
    n = features.shape[0]
    hh = Wh.shape[1]
    xproj = features @ Wx.T + (bx + bh)
    WhT = Wh.T.copy()
    lvl = np.zeros(n, np.int64)
    p = parent_idx
    for t in range(n):
        pt = p[t]
        lvl[t] = 0 if (pt < 0 or pt >= t) else lvl[pt] + 1
    hidden = np.zeros((n, hh), np.float32)
    c_all = np.zeros((n, hh), np.float32)
    for lv in range(int(lvl.max()) + 1):
        nodes = np.where(lvl == lv)[0]
        pn = p[nodes]
        hp = np.where((pn >= 0)[:, None],
                      np.where((pn < nodes)[:, None], hidden[pn], 0.0),
                      root_h)
        cp = np.where((pn >= 0)[:, None],
                      np.where((pn < nodes)[:, None], c_all[pn], 0.0),
                      root_c)
        h, c = _lstm_batch(xproj[nodes], hp.astype(np.float32),
                           cp.astype(np.float32), WhT)
        hidden[nodes] = h
        c_all[nodes] = c
    return hidden


# ------------------------------------------------------------ device program
def _build_program(loop_n=None, include_a=True, include_b=True):
    import concourse.bass as bass
    import concourse.mybir as mybir
    import concourse.tile as tile
    from concourse.masks import make_identity
    from contextlib import ExitStack

    F32 = mybir.dt.float32
    BF16 = mybir.dt.bfloat16
    AF = mybir.ActivationFunctionType

    nc = bass.Bass(target_bir_lowering=False)

    split = not (include_a and include_b)
    if include_a:
        feats = nc.declare_dram_parameter("feats", [16, 128, 1024], BF16, isOutput=False)
        wxblk = nc.declare_dram_parameter("wxblk", [8, 128, 16 * 512], BF16, isOutput=False)
        biasv = nc.declare_dram_parameter("biasv", [G], BF16, isOutput=False)
    if include_b:
        whtd = nc.declare_dram_parameter("wht_bf", [H, G], BF16, isOutput=False)
        hparT7 = nc.declare_dram_parameter("hparT7", [128, 8, 16], BF16, isOutput=False)
        cp7d = nc.declare_dram_parameter("cp7", [16, H], BF16, isOutput=False)
        hid = nc.declare_dram_parameter("hid", [NLOC, H], BF16, isOutput=True)
        cout = nc.declare_dram_parameter("cout", [1, H], BF16, isOutput=True)
    xpc_param = None
    if split:
        xpc_param = [[nc.declare_dram_parameter(
            f"xpc{j}_{f}", [128, G // 2], BF16,
            isOutput=(include_a and not include_b)) for f in range(2)]
            for j in range(8)]

    with tile.TileContext(nc) as tc:
        with ExitStack() as stack:
            ep = stack.enter_context
            persist = ep(tc.tile_pool(name="persist", bufs=1))
            psA = ep(tc.tile_pool(name="psA", bufs=4, space="PSUM"))
            psB = ep(tc.tile_pool(name="psB", bufs=2, space="PSUM"))
            psT = ep(tc.tile_pool(name="psT", bufs=2, space="PSUM"))
            pwx = ep(tc.tile_pool(name="pwx", bufs=3))
            pstA = ep(tc.tile_pool(name="pstA", bufs=2))
            pxps = ep(tc.tile_pool(name="pxps", bufs=2))
            pxpl = ep(tc.tile_pool(name="pxpl", bufs=2))
            phpar = ep(tc.tile_pool(name="phpar", bufs=2))
            pc = ep(tc.tile_pool(name="pc", bufs=2))
            pc12 = ep(tc.tile_pool(name="pc12", bufs=2))
            pcp = ep(tc.tile_pool(name="pcp", bufs=2))
            pg = ep(tc.tile_pool(name="pg", bufs=5))
            ph = ep(tc.tile_pool(name="ph", bufs=2))
            pdram = ep(tc.tile_pool(name="pdram", bufs=1, space="DRAM"))

            # per-chunk xproj scratch, split by eltwise column family:
            # family f holds col-blocks n8 with n8%2==f (gates gi at f*512+gi*1024)
            if xpc_param is not None:
                xpc = xpc_param
            else:
                xpc = [[pdram.tile([128, G // 2], BF16, name=f"xpc{j}_{f}")
                        for f in range(2)] for j in range(8)]

            def emit_body(iv=None):
                # startup-critical loads first: chunk-0 feature columns
                if include_a:
                    fta = persist.tile([128, 16, 128], BF16, name="fta")
                    nc.sync.dma_start(
                        fta[:], feats[:, :, 0:128].rearrange("k p c -> p k c"))
                    biasb = persist.tile([128, G], BF16, name="biasb")
                    bsrc = biasv[:]
                    nc.sync.dma_start(biasb[:], bass.AP(tensor=bsrc.tensor, offset=bsrc.offset,
                                                        ap=[[0, 128], bsrc.ap[0]]))
                if include_b:
                    ident = persist.tile([128, 128], F32, name="ident")
                    make_identity(nc, ident[:])
                    identb = persist.tile([128, 128], BF16, name="identb")
                    nc.vector.tensor_copy(identb[:], ident[:])
                    hp7 = persist.tile([128, 8, 16], BF16, name="hp7")
                    nc.sync.dma_start(hp7[:], hparT7[:])

                def emit_a_chunks(mcs, pfx, ftb=None):
                    for n8 in (0, 2, 4, 6, 1, 3, 5, 7):
                        wxq = []
                        for q in range(2):
                            wt = pwx.tile([128, 8, 512], BF16,
                                          name=f"wx{pfx}{n8}_{q}", tag="wx")
                            eng = nc.sync if q % 2 == 0 else nc.scalar
                            eng.dma_start(
                                wt[:],
                                wxblk[n8, :, q * 4096:(q + 1) * 4096]
                                .rearrange("p (k c) -> p k c", c=512))
                            wxq.append(wt)
                        for mc in mcs:
                            ps = psA.tile([128, 512], F32, name=f"pa{pfx}{n8}_{mc}",
                                          tag="psA")
                            for k in range(16):
                                src = (fta[:, k, :] if mc == 0 else
                                       ftb[:, k, (mc - 1) * 128:mc * 128])
                                nc.tensor.matmul(
                                    ps[:], src, wxq[k // 8][:, k % 8, :],
                                    start=(k == 0), stop=(k == 15))
                            st = pstA.tile([128, 512], BF16, name=f"st{pfx}{n8}_{mc}",
                                           tag="stA")
                            nc.vector.tensor_add(st[:], ps[:],
                                                 biasb[:, n8 * 512:(n8 + 1) * 512])
                            eng = nc.sync if (n8 + mc) % 2 == 0 else nc.scalar
                            gi, f = n8 // 2, n8 % 2
                            eng.dma_start(xpc[mc][f][:, gi * 512:(gi + 1) * 512],
                                          st[:])

                if include_a:
                    # ---- A: one n8-major sweep over all 8 chunks (Wx streamed
                    # once); family order (0,2,4,6,1,3,5,7) completes family-0
                    # xproj at the halfway point so B's h2=0 work can overlap
                    # A's natural (DMA-paced) PE gaps.
                    ftb = persist.tile([128, 16, 896], BF16, name="ftb")
                    nc.scalar.dma_start(
                        ftb[:], feats[:, :, 128:1024].rearrange("k p c -> p k c"))
                    emit_a_chunks(list(range(8)), "m", ftb)
                if include_b:
                    # ---- Wh slabs (B's recurrence unblocks per-slab)
                    whtb = []
                    for kc in range(8):
                        wt = persist.tile([128, G], BF16, name=f"whtb{kc}")
                        eng = nc.sync if kc % 2 == 0 else nc.scalar
                        eng.dma_start(wt[:], whtd[kc * 128:(kc + 1) * 128, :])
                        whtb.append(wt)
                if not include_b:
                    return

                # ---- B: levels 7..12
                hpar_cur = hp7
                c_prev = None
                for (lvl, nd, off) in LEVELS:
                    nch = max(1, (nd + 127) // 128)
                    if lvl == 12:
                        c_tiles = [pc12.tile([128, H], BF16, name=f"c12_{mc}",
                                             tag="c12") for mc in range(nch)]
                    else:
                        c_new = pc.tile([128, max(1, nch), H], BF16,
                                        name=f"c{lvl}", tag="c")
                    xp_small = None
                    if lvl <= 9:
                        xp_small = [pxps.tile([128, G // 2], BF16,
                                              name=f"xps{lvl}_{f}", tag="xps")
                                    for f in range(2)]
                        for f in range(2):
                            eng = nc.sync if f == 0 else nc.scalar
                            eng.dma_start(
                                xp_small[f][0:nd, :],
                                xpc[0][f][off + SHIFT:off + SHIFT + nd, :])
                    hpar_next = None
                    if lvl < 12:
                        hpar_next = phpar.tile([128, 8, 2 * nd], BF16,
                                               name=f"hp{lvl + 1}", tag="hpar")
                    for mc in range(nch):
                        mm = min(128, nd - mc * 128)
                        c0 = mc * 128
                        if lvl <= 9:
                            xp_t = xp_small
                        else:
                            j = (1 << (lvl - 10)) + mc
                            xp_t = [pxpl.tile([128, G // 2], BF16,
                                              name=f"xpl{lvl}_{mc}_{f}", tag="xpl")
                                    for f in range(2)]
                            for f in range(2):
                                eng = nc.sync if (mc + f) % 2 == 0 else nc.scalar
                                eng.dma_start(xp_t[f][:], xpc[j][f][:])
                        if lvl == 7:
                            cp_t = pcp.tile([128, H], BF16, name="cp7t", tag="cp")
                            nc.sync.dma_start(cp_t[0:16, :], cp7d[:])
                        else:
                            cp_t = pcp.tile([128, H], BF16, name=f"cp{lvl}_{mc}",
                                            tag="cp")
                            pchunk, r0, cnt = mc // 2, (mc * 64) % 128, mm // 2
                            nc.sync.dma_start(cp_t[0:mm:2, :],
                                              c_prev[r0:r0 + cnt, pchunk, :])
                            nc.sync.dma_start(cp_t[1:mm:2, :],
                                              c_prev[r0:r0 + cnt, pchunk, :])
                        hch = ph.tile([128, H], BF16, name=f"h{lvl}_{mc}", tag="h")
                        for h2 in range(2):
                            cs = h2 * 512
                            # two 2-bank subgroups: {i,u} then {f,o}
                            gs = {}
                            for sub, pair in enumerate(("iu", "fo")):
                                pss = {}
                                for gate in pair:
                                    pss[gate] = psB.tile(
                                        [128, 512], F32,
                                        name=f"g{lvl}_{mc}_{h2}_{gate}", tag="psB")
                                for k in range(8):
                                    for gate in pair:
                                        gi = "iofu".index(gate)
                                        nc.tensor.matmul(
                                            pss[gate][:mm, :],
                                            hpar_cur[:, k, c0:c0 + mm],
                                            whtb[k][:, gi * 1024 + cs:
                                                    gi * 1024 + cs + 512],
                                            start=(k == 0), stop=(k == 7))
                                for gate in pair:
                                    gi = "iofu".index(gate)
                                    g = pg.tile([128, 512], BF16,
                                                name=f"g{lvl}_{mc}_{gate}{h2}",
                                                tag="g")
                                    nc.vector.tensor_add(
                                        g[:mm, :], pss[gate][:mm, :],
                                        xp_t[h2][:mm, gi * 512:(gi + 1) * 512])
                                    nc.scalar.activation(
                                        out=g[:mm, :], in_=g[:mm, :],
                                        func=AF.Tanh if gate == "u" else AF.Sigmoid)
                                    gs[gate] = g
                            t1 = pg.tile([128, 512], BF16, name=f"t1{lvl}_{mc}_{h2}",
                                         tag="g")
                            nc.vector.tensor_mul(t1[:mm, :], gs["i"][:mm, :],
                                                 gs["u"][:mm, :])
                            t2 = pg.tile([128, 512], BF16, name=f"t2{lvl}_{mc}_{h2}",
                                         tag="g")
                            nc.vector.tensor_mul(t2[:mm, :], gs["f"][:mm, :],
                                                 cp_t[:mm, cs:cs + 512])
                            if lvl == 12:
                                cdst = c_tiles[mc][:mm, cs:cs + 512]
                            else:
                                cdst = c_new[:mm, mc, cs:cs + 512]
                            nc.vector.tensor_add(cdst, t1[:mm, :], t2[:mm, :])
                            tct = pg.tile([128, 512], BF16, name=f"tc{lvl}_{mc}_{h2}",
                                          tag="g")
                            nc.scalar.activation(out=tct[:mm, :], in_=cdst,
                                                 func=AF.Tanh)
                            nc.vector.tensor_mul(hch[:mm, cs:cs + 512],
                                                 gs["o"][:mm, :], tct[:mm, :])
                            if hpar_next is not None:
                                for k in range(h2 * 4, h2 * 4 + 4):
                                    pT = psT.tile([128, 128], BF16,
                                                  name=f"pT{lvl}_{mc}_{k}", tag="psT")
                                    nc.tensor.transpose(
                                        pT[:, :mm], hch[:mm, k * 128:(k + 1) * 128],
                                        identb[:mm, :mm])
                                    src = pT[:, :mm]
                                    rap = bass.AP(tensor=src.tensor, offset=src.offset,
                                                  ap=[src.ap[0], src.ap[1], [0, 2]])
                                    dst = hpar_next[:, k, 2 * c0:2 * (c0 + mm)]
                                    nc.vector.tensor_copy(
                                        dst.rearrange("p (n two) -> p n two", two=2),
                                        rap)
                        eng = nc.sync if mc % 2 == 0 else nc.scalar
                        eng.dma_start(hid[off + c0:off + c0 + mm, :], hch[:mm, :])
                        if lvl == 12 and mc == 0:
                            nc.sync.dma_start(cout[0:1, :], c_tiles[0][0:1, :])
                    if lvl < 12:
                        c_prev = c_new
                    hpar_cur = hpar_next

            if loop_n is not None and loop_n > 1:
                with tc.For_i(0, loop_n, 1):
                    emit_body()
            else:
                emit_body()

    _split_excess_waits(nc)
    return nc


def _split_excess_waits(nc, max_waits=1):
    """Walrus build rejects >1 sem wait per hardware instruction; spill the
    excess onto same-engine NoOps placed immediately before."""
    import concourse.mybir as mybir
    ctr = 0
    for fn in nc.m.functions:
        for bb in fn.blocks:
            il = bb.instructions
            if not any(i.sync_info is not None and i.sync_info.on_wait
                       and len(i.sync_info.on_wait) > max_waits for i in il):
                continue
            new_list = []
            for inst in il:
                si = inst.sync_info
                if si is not None and si.on_wait and len(si.on_wait) > max_waits:
                    waits = list(si.on_wait)
                    for w in waits[:-max_waits]:
                        ctr += 1
                        nop = mybir.InstNoOp(name=f"waitspill_{ctr}", ins=[], outs=[])
                        nop.engine = inst.engine
                        nop.sync_info = mybir.SyncInfo(on_wait=[w], on_update=[])
                        try:
                            nc.register_instruction(nop, overwrite=True)
                        except Exception:
                            pass
                        new_list.append(nop)
                    si.on_wait = waits[-max_waits:]
                new_list.append(inst)
            bb.instructions[:] = new_list
    return ctr


# ------------------------------------------------------------------ host side
def _host_prefix(features, Wx, bx, Wh, bh, root_c, root_h):
    """Nodes 0..126 (levels 0..6) on the host. Returns (h127, c127)."""
    xp = features[0:127] @ Wx.T + (bx + bh)
    WhT = np.ascontiguousarray(Wh.T)
    h = np.zeros((127, H), np.float32)
    c = np.zeros((127, H), np.float32)
    for d in range(7):
        i0, n = (1 << d) - 1, 1 << d
        if d == 0:
            hp = root_h.reshape(1, H).astype(np.float32)
            cp = root_c.reshape(1, H).astype(np.float32)
        else:
            par = (np.arange(i0, i0 + n) - 1) // 2
            hp, cp = h[par], c[par]
        h[i0:i0 + n], c[i0:i0 + n] = _lstm_batch(xp[i0:i0 + n], hp, cp, WhT)
    return h, c


def _local_rows(core):
    rows = []
    for (d, s, off) in LEVELS:
        g0 = (1 << d) - 1 + core * s
        rows.append(np.arange(g0, g0 + s))
    return np.concatenate(rows)  # 1008 global node ids, local order


def _stage_shared(Wx, bx, bh, Wh):
    import ml_dtypes
    bf = ml_dtypes.bfloat16
    w = Wx.T.reshape(16, 128, 8, 512)          # [k, p, n, c]
    wxblk = np.ascontiguousarray(
        w.transpose(2, 1, 0, 3).reshape(8, 128, 16 * 512)).astype(bf)
    return {
        "wxblk": wxblk,
        "biasv": (bx + bh).astype(bf),
        "wht_bf": np.ascontiguousarray(Wh.T).astype(bf),
    }


def _make_core_inputs(features, h127, c127, shared):
    import ml_dtypes
    bf = ml_dtypes.bfloat16
    maps = []
    for k in range(NCORES):
        rows = _local_rows(k)
        S = np.zeros((1024, F), np.float32)
        S[SHIFT:] = features[rows]
        ftk = np.ascontiguousarray(S.T.reshape(16, 128, 1024)).astype(bf)
        # parents of this core's level-7 nodes: level-6 nodes 63+8k .. 63+8k+8
        hsel = h127[63 + 8 * k:63 + 8 * k + 8]          # [8, H]
        csel = c127[63 + 8 * k:63 + 8 * k + 8]
        hT = hsel.T.reshape(8, 128, 8).transpose(1, 0, 2)  # [128, kc, 8]
        m = dict(shared)
        m["feats"] = ftk
        m["hparT7"] = np.ascontiguousarray(np.repeat(hT, 2, axis=2)).astype(bf)
        m["cp7"] = np.ascontiguousarray(np.repeat(csel, 2, axis=0)).astype(bf)
        maps.append(m)
    return maps


def _assemble(features, Wx, bx, bh, Wh, h127, results):
    hidden = np.empty((N, H), np.float32)
    hidden[0:127] = h127
    for k in range(NCORES):
        hk = results[k]["hid"].astype(np.float32)
        for (d, s, off) in LEVELS:
            g0 = (1 << d) - 1 + k * s
            hidden[g0:g0 + s] = hk[off:off + s]
    # node 8191 on host: parent = 4095 (core 0, level-12 chunk 0 row 0)
    c4095 = results[0]["cout"].astype(np.float32).reshape(1, H)
    h4095 = hidden[4095].reshape(1, H)
    xp = features[8191:8192] @ Wx.T + (bx + bh)
    h, _ = _lstm_batch(xp, h4095, c4095, np.ascontiguousarray(Wh.T))
    hidden[8191] = h[0]
    return hidden


def _expected_parent_idx():
    t = np.arange(N)
    p = (t - 1) // 2
    p[0] = -1
    return p.astype(np.int64)


def kernel(features, Wx, bx, Wh, bh, parent_idx, root_c, root_h):
    features = np.ascontiguousarray(np.asarray(features, dtype=np.float32))
    Wx = np.ascontiguousarray(np.asarray(Wx, dtype=np.float32))
    bx = np.asarray(bx, dtype=np.float32)
    Wh = np.ascontiguousarray(np.asarray(Wh, dtype=np.float32))
    bh = np.asarray(bh, dtype=np.float32)
    parent_idx = np.asarray(parent_idx)
    root_c = np.asarray(root_c, dtype=np.float32)
    root_h = np.asarray(root_h, dtype=np.float32)

    if (features.shape != (N, F) or Wx.shape != (G, F) or Wh.shape != (G, H)
            or not np.array_equal(parent_idx.astype(np.int64).ravel(),
                                  _expected_parent_idx())):
        return _numpy_fallback(features, Wx, bx, Wh, bh,
                               parent_idx.astype(np.int64).ravel(),
                               root_c.reshape(1, -1), root_h.reshape(1, -1))

    try:
        return _device_kernel(features, Wx, bx, Wh, bh, root_c, root_h)
    except Exception as e:
        sys.stderr.write(f"[kernel] device path failed ({type(e).__name__}: {e}); retrying\n")
        try:
            return _device_kernel(features, Wx, bx, Wh, bh, root_c, root_h)
        except Exception as e2:
            sys.stderr.write(f"[kernel] device retry failed ({type(e2).__name__}: {e2}); "
                             "using numpy fallback\n")
            return _numpy_fallback(features, Wx, bx, Wh, bh,
                                   parent_idx.astype(np.int64).ravel(),
                                   root_c.reshape(1, -1), root_h.reshape(1, -1))


def _device_kernel(features, Wx, bx, Wh, bh, root_c, root_h):
    from concourse.bass_utils import run_bass_kernel_spmd

    h127, c127 = _host_prefix(features, Wx, bx, Wh, bh, root_c, root_h)

    if "main" not in _prog_cache:
        _prog_cache["main"] = _build_program()
    nc = _prog_cache["main"]

    shared = _stage_shared(Wx, bx, bh, Wh)
    in_maps = _make_core_inputs(features, h127, c127, shared)
    results = run_bass_kernel_spmd(nc, in_maps, list(range(NCORES))).results
    return _assemble(features, Wx, bx, bh, Wh, h127, results)
